# revision 46
# baseline (speedup 1.0000x reference)
"""Trainium2 Bass kernel for ArccosHessianCalculator.

Math: for each batch element b (z1, z2 are [B, D] with D = 128):
  a = 1/|z1|, bb = 1/|z2|, c = cos = <z1u, z2u>
  Each Hessian block H_k is a rank-2 outer product plus a diagonal term:
      H_k(b) = z1 * r0_k(b)^T + z2 * r1_k(b)^T + diag-part
  where r0/r1 are per-element linear combinations of z1, z2 (all the
  normalization / cosine scale factors folded into the coefficients):
      k=0 (H11): r0 = -3c*a^4*z1 + a^3 b*z2          r1 = a^3 b*z1
      k=1 (H12): r0 = a^3 b*z1                        r1 = -c*a^2 b^2*z1 + a b^3*z2
      k=2 (H22): r0 = a b^3*z2                        r1 = a b^3*z1 - 3c*b^4*z2
  The (full, final) diagonals are computed separately in closed form and
  spliced in with a predicated copy against an identity mask.

Mapping to the chip (per core, batch shard of 512):
  - TensorE: one K=2 matmul per element, lhsT = [z1(b); z2(b)] ([2,128]),
    rhs = [r0 | r1] blocks ([2, 384]), streamed as float32r (1 cyc/row).
    Operands live at partition offsets {0,32} (tile_position rule; 64 legal
    too, quadrant 96 unusable).
  - ScalarE: PSUM -> SBUF staging copy.
  - VectorE: stats + rhs coefficient builds + diagonal splice
    (copy_predicated with an eye mask and a broadcast diagonal column).
  - DMA: batched 1-2MB output writes, one per (staging group, k).
The per-group stats work is software-pipelined: group g+1's stats are
emitted in four slices interleaved between group g's element chunks, so the
DVE/ACT stats burst never makes one chunk's compute exceed its DMA window.
Output per core: [3, 512, 128, 128] f32 (~100MB) -> DMA-bound overall.
"""

import numpy as np
from contextlib import ExitStack

import concourse.bass as bass
import concourse.tile as tile
from concourse import bacc, mybir
from concourse.bass_utils import run_bass_kernel_spmd

N_CORES = 8
B_FULL = 4096
D = 128
B_SH = B_FULL // N_CORES  # 512 batch elements per core
P = 128                   # SBUF partitions
KD = 3 * D                # 384: three H blocks side by side
F = 16                    # elements per partition-group row in ZI/RI tiles
G = 32                    # elements per gather chunk
GROUPS = B_SH // P        # 4 stats groups of 128 elements
NCH = P // G              # 4 chunks per group

f32 = mybir.dt.float32
f32r = mybir.dt.float32r
i32 = mybir.dt.int32


class _Pools:
    pass


def _make_pools(ctx, tc):
    p = _Pools()
    p.const = ctx.enter_context(tc.tile_pool(name="const", bufs=1))
    p.zg = ctx.enter_context(tc.tile_pool(name="zg", bufs=3))
    p.work = ctx.enter_context(tc.tile_pool(name="work", bufs=2))
    p.stat = ctx.enter_context(tc.tile_pool(name="stat", bufs=3))
    p.rpool = ctx.enter_context(tc.tile_pool(name="rpool", bufs=3))
    p.dpool = ctx.enter_context(tc.tile_pool(name="dpool", bufs=3))
    p.zi = ctx.enter_context(tc.tile_pool(name="zi", bufs=2))
    p.ri = ctx.enter_context(tc.tile_pool(name="ri", bufs=2))
    p.stage = ctx.enter_context(tc.tile_pool(name="stage", bufs=2))
    p.mmp = ctx.enter_context(tc.tile_pool(name="mmp", bufs=6, space="PSUM"))
    p.tpp = ctx.enter_context(tc.tile_pool(name="tpp", bufs=2, space="PSUM"))
    return p


def _emit_consts(p, nc):
    A = mybir.AluOpType
    colidx_i = p.const.tile([P, D], i32)
    nc.gpsimd.iota(colidx_i[:], [[1, D]], base=0, channel_multiplier=0)
    rowidx_i = p.const.tile([P, 1], i32)
    nc.gpsimd.iota(rowidx_i[:], [[0, 1]], base=0, channel_multiplier=1)
    colidx = p.const.tile([P, D], f32)
    nc.vector.tensor_copy(colidx[:], colidx_i[:])
    rowidx = p.const.tile([P, 1], f32)
    nc.vector.tensor_copy(rowidx[:], rowidx_i[:])
    eye = p.const.tile([P, D], f32)
    nc.vector.tensor_scalar(eye[:], colidx[:], rowidx[:], None, A.is_equal)
    # integer mask for copy_predicated (hw requires an int mask dtype)
    eyem = p.const.tile([P, D], mybir.dt.uint8)
    nc.vector.tensor_scalar(eyem[:], colidx[:], rowidx[:], None, A.is_equal)
    eye3 = p.const.tile([P, KD], mybir.dt.uint8)
    for k in range(3):
        nc.vector.tensor_copy(eye3[:, k * D:(k + 1) * D], eyem[:])
    # warm the ACT Sqrt function table while the first input loads are in
    # flight, so group 0's norm chain doesn't pay the ~1.3us table load
    warm = p.const.tile([P, 1], f32)
    nc.scalar.sqrt(warm[:], rowidx[:])
    p.eye, p.eye3 = eye, eye3


def _stats_phase0(p, nc, z1, z2, grp, use_f32r):
    """Loads, norms/cosine and the per-element scalar coefficient chain."""
    A = mybir.AluOpType
    st = {}
    b0 = grp * P
    # later groups' input loads go via gpsimd so they can't head-of-line
    # block group 0's gathers/output writes on the sync ring at startup
    ldma = nc.sync if grp == 0 else nc.gpsimd
    z1g = p.zg.tile([P, D], f32, tag="z1g", name=f"z1g_{grp}")
    ldma.dma_start(z1g[:], z1[b0:b0 + P, :])
    z2g = p.zg.tile([P, D], f32, tag="z2g", name=f"z2g_{grp}")
    ldma.dma_start(z2g[:], z2[b0:b0 + P, :])

    def wt(tag):
        return p.work.tile([P, D], f32, tag=tag, name=f"w_{tag}_{grp}")

    def sv(tag):
        return p.stat.tile([P, 1], f32, tag=tag, name=f"sv_{tag}_{grp}")

    v1z, v2z, wz = wt("v1z"), wt("v2z"), wt("wz")
    nc.vector.tensor_mul(v1z[:], z1g[:], z1g[:])
    nc.vector.tensor_mul(v2z[:], z2g[:], z2g[:])
    nc.vector.tensor_mul(wz[:], z1g[:], z2g[:])

    s1, s2, dot = sv("s1"), sv("s2"), sv("dot")
    nc.vector.reduce_sum(s1[:], v1z[:], axis=mybir.AxisListType.X)
    nc.vector.reduce_sum(s2[:], v2z[:], axis=mybir.AxisListType.X)
    nc.vector.reduce_sum(dot[:], wz[:], axis=mybir.AxisListType.X)
    n1, n2 = sv("n1"), sv("n2")
    nc.scalar.sqrt(n1[:], s1[:])
    nc.scalar.sqrt(n2[:], s2[:])
    a, bb = sv("a"), sv("bb")
    nc.vector.reciprocal(a[:], n1[:])
    nc.vector.reciprocal(bb[:], n2[:])
    a2, b2, ab, c = sv("a2"), sv("b2"), sv("ab"), sv("c")
    nc.vector.tensor_mul(a2[:], a[:], a[:])
    nc.vector.tensor_mul(b2[:], bb[:], bb[:])
    nc.vector.tensor_mul(ab[:], a[:], bb[:])
    nc.vector.tensor_mul(c[:], dot[:], ab[:])
    m3c, mc = sv("m3c"), sv("mc")
    nc.vector.tensor_scalar(m3c[:], c[:], -3.0, None, A.mult)
    nc.vector.tensor_scalar(mc[:], c[:], -1.0, None, A.mult)
    A3B, AB3, A4, B4, A2B2 = sv("A3B"), sv("AB3"), sv("A4"), sv("B4"), sv("A2B2")
    nc.vector.tensor_mul(A3B[:], a2[:], ab[:])
    nc.vector.tensor_mul(AB3[:], b2[:], ab[:])
    nc.vector.tensor_mul(A4[:], a2[:], a2[:])
    nc.vector.tensor_mul(B4[:], b2[:], b2[:])
    nc.vector.tensor_mul(A2B2[:], ab[:], ab[:])
    m3cA4, m3cB4, mcA2B2, mcab = sv("m3cA4"), sv("m3cB4"), sv("mcA2B2"), sv("mcab")
    nc.vector.tensor_mul(m3cA4[:], A4[:], m3c[:])
    nc.vector.tensor_mul(m3cB4[:], B4[:], m3c[:])
    nc.vector.tensor_mul(mcA2B2[:], A2B2[:], mc[:])
    nc.vector.tensor_mul(mcab[:], ab[:], mc[:])

    # rounded copies of z1/z2 for the matmul lhsT gathers
    mmdt = f32r if use_f32r else f32
    z1r = p.zg.tile([P, D], mmdt, tag="z1r", name=f"z1r_{grp}")
    nc.vector.tensor_copy(z1r[:], z1g[:])
    z2r = p.zg.tile([P, D], mmdt, tag="z2r", name=f"z2r_{grp}")
    nc.vector.tensor_copy(z2r[:], z2g[:])

    st.update(z1g=z1g, z2g=z2g, v1z=v1z, v2z=v2z, wz=wz, a2=a2, b2=b2, ab=ab,
              c=c, m3c=m3c, A3B=A3B, AB3=AB3, m3cA4=m3cA4, m3cB4=m3cB4,
              mcA2B2=mcA2B2, mcab=mcab, z1r=z1r, z2r=z2r, wt=wt)
    return st


def _stats_phase1(p, nc, st, grp, use_f32r):
    """rhs rows R0, R1 [128b, 384] in float32r (rounded on DVE write)."""
    A = mybir.AluOpType
    mmdt = f32r if use_f32r else f32
    z1g, z2g, wt = st["z1g"], st["z2g"], st["wt"]
    A3B, AB3 = st["A3B"], st["AB3"]
    R0 = p.rpool.tile([P, KD], mmdt, tag="R0", name=f"R0_{grp}")
    R1 = p.rpool.tile([P, KD], mmdt, tag="R1", name=f"R1_{grp}")
    t0 = wt("t0")
    # k=0 (H11): r0 = m3cA4*z1 + A3B*z2 ; r1 = A3B*z1
    nc.vector.tensor_scalar(t0[:], z2g[:], A3B[:], None, A.mult)
    nc.vector.scalar_tensor_tensor(
        R0[:, 0:D], z1g[:], st["m3cA4"][:], t0[:], A.mult, A.add)
    nc.vector.tensor_scalar(R1[:, 0:D], z1g[:], A3B[:], None, A.mult)
    # k=1 (H12): r0 = A3B*z1 ; r1 = mcA2B2*z1 + AB3*z2
    nc.vector.tensor_scalar(R0[:, D:2 * D], z1g[:], A3B[:], None, A.mult)
    t1 = wt("t1")
    nc.vector.tensor_scalar(t1[:], z2g[:], AB3[:], None, A.mult)
    nc.vector.scalar_tensor_tensor(
        R1[:, D:2 * D], z1g[:], st["mcA2B2"][:], t1[:], A.mult, A.add)
    # k=2 (H22): r0 = AB3*z2 ; r1 = AB3*z1 + m3cB4*z2
    nc.vector.tensor_scalar(R0[:, 2 * D:3 * D], z2g[:], AB3[:], None, A.mult)
    t2 = wt("t2")
    nc.vector.tensor_scalar(t2[:], z2g[:], st["m3cB4"][:], None, A.mult)
    nc.vector.scalar_tensor_tensor(
        R1[:, 2 * D:3 * D], z1g[:], AB3[:], t2[:], A.mult, A.add)
    st.update(R0=R0, R1=R1)


def _stats_phase2(p, nc, st, grp):
    """Final diagonal values, batch-major [128b, 128i]."""
    A = mybir.AluOpType
    wt = st["wt"]
    v1z, v2z, wz = st["v1z"], st["v2z"], st["wz"]
    a2, b2, ab, c, m3c = st["a2"], st["b2"], st["ab"], st["c"], st["m3c"]
    twoabw = wt("twoabw")
    nc.vector.tensor_scalar(twoabw[:], wz[:], ab[:], 2.0, A.mult, A.mult)
    # d11 = a2*(c + 2ab*wz + m3c*a2*v1z)
    u1, u2 = wt("u1"), wt("u2")
    nc.vector.tensor_scalar(u1[:], v1z[:], a2[:], m3c[:], A.mult, A.mult)
    nc.vector.tensor_add(u2[:], u1[:], twoabw[:])
    d11 = p.dpool.tile([P, D], f32, tag="d11", name=f"d11_{grp}")
    nc.vector.tensor_scalar(d11[:], u2[:], c[:], a2[:], A.add, A.mult)
    # d22 = b2*(c + 2ab*wz + m3c*b2*v2z)
    u3, u4 = wt("u3"), wt("u4")
    nc.vector.tensor_scalar(u3[:], v2z[:], b2[:], m3c[:], A.mult, A.mult)
    nc.vector.tensor_add(u4[:], u3[:], twoabw[:])
    d22 = p.dpool.tile([P, D], f32, tag="d22", name=f"d22_{grp}")
    nc.vector.tensor_scalar(d22[:], u4[:], c[:], b2[:], A.add, A.mult)
    # d12 = ab*(a2*v1z + b2*v2z + mcab*wz - 1)
    w1, w2, w3 = wt("w1"), wt("w2"), wt("w3")
    nc.vector.tensor_scalar(w1[:], v1z[:], a2[:], None, A.mult)
    nc.vector.scalar_tensor_tensor(w2[:], v2z[:], b2[:], w1[:], A.mult, A.add)
    nc.vector.scalar_tensor_tensor(w3[:], wz[:], st["mcab"][:], w2[:],
                                   A.mult, A.add)
    d12 = p.dpool.tile([P, D], f32, tag="d12", name=f"d12_{grp}")
    nc.vector.tensor_scalar(d12[:], w3[:], -1.0, ab[:], A.add, A.mult)
    st.update(d11=d11, d12=d12, d22=d22)


def _stats_phase3(p, nc, st, grp):
    """Transpose diagonals into [128i, 3*128b]."""
    diagT = p.dpool.tile([P, KD], f32, tag="diagT", name=f"diagT_{grp}")
    for k, dk in enumerate([st["d11"], st["d12"], st["d22"]]):
        pt = p.tpp.tile([P, D], f32, tag="tp", name=f"tp_{grp}_{k}")
        nc.tensor.transpose(pt[:], dk[:], p.eye[:])
        nc.scalar.copy(diagT[:, k * D:(k + 1) * D], pt[:])
    st.update(diagT=diagT)


def _emit_chunk(p, nc, st, out, grp, ch, use_f32r):
    """Gathers + 32 elements (matmul/copy/diag-splice) + output DMAs."""
    mmdt = f32r if use_f32r else f32
    b0 = grp * P
    e0 = b0 + ch * G          # global element base for this chunk
    q0 = ch * G               # within-group base
    # Gather F batch rows into one partition row per (group, operand).
    # Out is a single-partition free-linear run; in is a plain slice —
    # stream orders match (b-major), dma_start only checks total size.
    ZI = p.zi.tile([P, F * D], mmdt, tag="ZI", name=f"ZI_{grp}_{ch}")
    RI = p.ri.tile([P, F * KD], mmdt, tag="RI", name=f"RI_{grp}_{ch}")
    # gathers ride the (otherwise idle) gpsimd SWDGE path so the big output
    # writes on the sync HWDGE ring can't head-of-line block the next
    # chunk's operands; the very first chunk uses the still empty sync ring
    dmae = nc.sync if (grp == 0 and ch == 0) else nc.gpsimd
    z1r, z2r, R0, R1 = st["z1r"], st["z2r"], st["R0"], st["R1"]
    for g in range(2):
        qs = q0 + g * F
        dmae.dma_start(ZI[32 * g:32 * g + 1, :], z1r[qs:qs + F, :])
        dmae.dma_start(ZI[32 * g + 1:32 * g + 2, :], z2r[qs:qs + F, :])
        dmae.dma_start(RI[32 * g:32 * g + 1, :], R0[qs:qs + F, :])
        dmae.dma_start(RI[32 * g + 1:32 * g + 2, :], R1[qs:qs + F, :])

    # staging sub-groups of GS elements: the first chunks use finer
    # granularity so the first output DMA launches earlier, and the last
    # chunk so the final drain is shorter
    # fine staging only at the very start (early first output DMA) and the
    # very end (short final drain); full chunks elsewhere leave the most
    # slack to absorb the interleaved stats phases without starving DMA
    ci = grp * NCH + ch
    if ci == 0 or ci == GROUPS * NCH - 1:
        GS = G // 4
    elif ci == 1:
        GS = G // 2
    else:
        GS = G
    diagT = st["diagT"]
    for sub in range(G // GS):
        STG = p.stage.tile([P, GS * KD], f32, tag="STG",
                           name=f"STG_{grp}_{ch}_{sub}")
        for s0 in range(GS):
            s = sub * GS + s0
            q = q0 + s            # element idx within group (0..127)
            g4, ff = s // F, s % F
            pp = 32 * g4
            lhsT = ZI[pp:pp + 2, ff * D:(ff + 1) * D]
            rhs = RI[pp:pp + 2, ff * KD:(ff + 1) * KD]
            pt = p.mmp.tile([P, KD], f32, tag="pt", name=f"pt_{grp}_{ch}_{s}")
            nc.tensor.matmul(pt[:], lhsT, rhs, start=True, stop=True)
            dst = STG[:, s0 * KD:(s0 + 1) * KD]
            nc.scalar.copy(dst, pt[:])
            # contiguous innermost j on dst/mask; data is a per-k column of
            # diagT broadcast along j (step-0 inner dim)
            datav = diagT[:].rearrange("p (k b) -> p k b", k=3)[
                :, :, q:q + 1].broadcast_to([P, 3, D])
            nc.vector.copy_predicated(dst, p.eye3[:], datav)
        bs = e0 + sub * GS
        stgv = STG[:].rearrange("p (e n) -> p e n", n=KD)
        for k in range(3):
            dram = out[k, bs:bs + GS, :, :].transpose([1, 0, 2])
            nc.sync.dma_start(dram, stgv[:, :, k * D:(k + 1) * D])


def _build_body(ctx, tc, z1, z2, out, use_f32r=True):
    nc = tc.nc
    p = _make_pools(ctx, tc)
    _emit_consts(p, nc)

    # group 0's stats run up front; each later group's stats are emitted in
    # slices between the previous group's chunks so the DVE/ACT burst is
    # amortized and no single chunk's compute exceeds its DMA drain window
    cur = _stats_phase0(p, nc, z1, z2, 0, use_f32r)
    _stats_phase1(p, nc, cur, 0, use_f32r)
    _stats_phase2(p, nc, cur, 0)
    _stats_phase3(p, nc, cur, 0)
    for grp in range(GROUPS):
        nxt = None
        for ch in range(NCH):
            _emit_chunk(p, nc, cur, out, grp, ch, use_f32r)
            if grp + 1 < GROUPS:
                if ch == 0:
                    nxt = _stats_phase0(p, nc, z1, z2, grp + 1, use_f32r)
                elif ch == 1:
                    _stats_phase1(p, nc, nxt, grp + 1, use_f32r)
                elif ch == 2:
                    _stats_phase2(p, nc, nxt, grp + 1)
                else:
                    _stats_phase3(p, nc, nxt, grp + 1)
        if nxt is not None:
            cur = nxt


def build_kernel(use_f32r=True):
    nc = bacc.Bacc("TRN2", target_bir_lowering=False, debug=False)
    z1 = nc.dram_tensor("z1", [B_SH, D], f32, kind="ExternalInput").ap()
    z2 = nc.dram_tensor("z2", [B_SH, D], f32, kind="ExternalInput").ap()
    out = nc.dram_tensor("out", [3, B_SH, D, D], f32, kind="ExternalOutput").ap()
    with tile.TileContext(nc) as tc:
        with ExitStack() as ctx:
            _build_body(ctx, tc, z1, z2, out, use_f32r=use_f32r)
    nc.compile()
    return nc


_NC_CACHE = None


def _get_nc():
    global _NC_CACHE
    if _NC_CACHE is None:
        _NC_CACHE = build_kernel()
    return _NC_CACHE


def kernel(z1, z2):
    nc = _get_nc()
    z1 = np.ascontiguousarray(np.asarray(z1, dtype=np.float32))
    z2 = np.ascontiguousarray(np.asarray(z2, dtype=np.float32))
    in_maps = [
        {"z1": z1[c * B_SH:(c + 1) * B_SH], "z2": z2[c * B_SH:(c + 1) * B_SH]}
        for c in range(N_CORES)
    ]
    res = run_bass_kernel_spmd(nc, in_maps, core_ids=list(range(N_CORES)))
    return np.concatenate([res.results[c]["out"] for c in range(N_CORES)], axis=1)


# revision 48
# speedup vs baseline: 1.0090x; 1.0090x over previous
"""Trainium2 Bass kernel for ArccosHessianCalculator.

Math: for each batch element b (z1, z2 are [B, D] with D = 128):
  a = 1/|z1|, bb = 1/|z2|, c = cos = <z1u, z2u>
  Each Hessian block H_k is a rank-2 outer product plus a diagonal term:
      H_k(b) = z1 * r0_k(b)^T + z2 * r1_k(b)^T + diag-part
  where r0/r1 are per-element linear combinations of z1, z2 (all the
  normalization / cosine scale factors folded into the coefficients):
      k=0 (H11): r0 = -3c*a^4*z1 + a^3 b*z2          r1 = a^3 b*z1
      k=1 (H12): r0 = a^3 b*z1                        r1 = -c*a^2 b^2*z1 + a b^3*z2
      k=2 (H22): r0 = a b^3*z2                        r1 = a b^3*z1 - 3c*b^4*z2
  The (full, final) diagonals are computed separately in closed form and
  spliced in with a predicated copy against an identity mask.

Mapping to the chip (per core, batch shard of 512):
  - TensorE: one K=2 matmul per element, lhsT = [z1(b); z2(b)] ([2,128]),
    rhs = [r0 | r1] blocks ([2, 384]), streamed as float32r (1 cyc/row).
    Operands live at partition offsets {0,32} (tile_position rule; 64 legal
    too, quadrant 96 unusable).
  - ScalarE: PSUM -> SBUF staging copy.
  - VectorE: stats + rhs coefficient builds + diagonal splice
    (copy_predicated with an eye mask and a broadcast diagonal column).
  - DMA: batched 1-2MB output writes, one per (staging group, k).
The per-group stats work is software-pipelined: group g+1's stats are
emitted in four slices interleaved between group g's element chunks, so the
DVE/ACT stats burst never makes one chunk's compute exceed its DMA window.
Output per core: [3, 512, 128, 128] f32 (~100MB) -> DMA-bound overall.
"""

import numpy as np
from contextlib import ExitStack

import concourse.bass as bass
import concourse.tile as tile
from concourse import bacc, mybir
from concourse.bass_utils import run_bass_kernel_spmd

N_CORES = 8
B_FULL = 4096
D = 128
B_SH = B_FULL // N_CORES  # 512 batch elements per core
P = 128                   # SBUF partitions
KD = 3 * D                # 384: three H blocks side by side
F = 16                    # elements per partition-group row in ZI/RI tiles
G = 32                    # elements per gather chunk
GROUPS = B_SH // P        # 4 stats groups of 128 elements
NCH = P // G              # 4 chunks per group

f32 = mybir.dt.float32
f32r = mybir.dt.float32r
i32 = mybir.dt.int32


class _Pools:
    pass


def _make_pools(ctx, tc):
    p = _Pools()
    p.const = ctx.enter_context(tc.tile_pool(name="const", bufs=1))
    p.zg = ctx.enter_context(tc.tile_pool(name="zg", bufs=3))
    p.work = ctx.enter_context(tc.tile_pool(name="work", bufs=2))
    p.stat = ctx.enter_context(tc.tile_pool(name="stat", bufs=3))
    p.rpool = ctx.enter_context(tc.tile_pool(name="rpool", bufs=3))
    p.dpool = ctx.enter_context(tc.tile_pool(name="dpool", bufs=3))
    p.zi = ctx.enter_context(tc.tile_pool(name="zi", bufs=2))
    p.ri = ctx.enter_context(tc.tile_pool(name="ri", bufs=2))
    p.stage = ctx.enter_context(tc.tile_pool(name="stage", bufs=2))
    p.mmp = ctx.enter_context(tc.tile_pool(name="mmp", bufs=6, space="PSUM"))
    p.tpp = ctx.enter_context(tc.tile_pool(name="tpp", bufs=2, space="PSUM"))
    return p


def _emit_consts(p, nc):
    A = mybir.AluOpType
    colidx_i = p.const.tile([P, D], i32)
    nc.gpsimd.iota(colidx_i[:], [[1, D]], base=0, channel_multiplier=0)
    rowidx_i = p.const.tile([P, 1], i32)
    nc.gpsimd.iota(rowidx_i[:], [[0, 1]], base=0, channel_multiplier=1)
    colidx = p.const.tile([P, D], f32)
    nc.vector.tensor_copy(colidx[:], colidx_i[:])
    rowidx = p.const.tile([P, 1], f32)
    nc.vector.tensor_copy(rowidx[:], rowidx_i[:])
    eye = p.const.tile([P, D], f32)
    nc.vector.tensor_scalar(eye[:], colidx[:], rowidx[:], None, A.is_equal)
    # integer mask for copy_predicated (hw requires an int mask dtype)
    eyem = p.const.tile([P, D], mybir.dt.uint8)
    nc.vector.tensor_scalar(eyem[:], colidx[:], rowidx[:], None, A.is_equal)
    eye3 = p.const.tile([P, KD], mybir.dt.uint8)
    for k in range(3):
        nc.vector.tensor_copy(eye3[:, k * D:(k + 1) * D], eyem[:])
    p.eye, p.eye3 = eye, eye3


def _stats_phase0(p, nc, z1, z2, grp, use_f32r):
    """Loads, norms/cosine and the per-element scalar coefficient chain."""
    A = mybir.AluOpType
    st = {}
    b0 = grp * P
    # later groups' input loads go via gpsimd so they can't head-of-line
    # block group 0's gathers/output writes on the sync ring at startup
    ldma = nc.sync if grp == 0 else nc.gpsimd
    z1g = p.zg.tile([P, D], f32, tag="z1g", name=f"z1g_{grp}")
    ldma.dma_start(z1g[:], z1[b0:b0 + P, :])
    z2g = p.zg.tile([P, D], f32, tag="z2g", name=f"z2g_{grp}")
    ldma.dma_start(z2g[:], z2[b0:b0 + P, :])

    def wt(tag):
        return p.work.tile([P, D], f32, tag=tag, name=f"w_{tag}_{grp}")

    def sv(tag):
        return p.stat.tile([P, 1], f32, tag=tag, name=f"sv_{tag}_{grp}")

    v1z, v2z, wz = wt("v1z"), wt("v2z"), wt("wz")
    nc.vector.tensor_mul(v1z[:], z1g[:], z1g[:])
    nc.vector.tensor_mul(v2z[:], z2g[:], z2g[:])
    nc.vector.tensor_mul(wz[:], z1g[:], z2g[:])

    s1, s2, dot = sv("s1"), sv("s2"), sv("dot")
    nc.vector.reduce_sum(s1[:], v1z[:], axis=mybir.AxisListType.X)
    nc.vector.reduce_sum(s2[:], v2z[:], axis=mybir.AxisListType.X)
    nc.vector.reduce_sum(dot[:], wz[:], axis=mybir.AxisListType.X)
    n1, n2 = sv("n1"), sv("n2")
    nc.scalar.sqrt(n1[:], s1[:])
    nc.scalar.sqrt(n2[:], s2[:])
    a, bb = sv("a"), sv("bb")
    nc.vector.reciprocal(a[:], n1[:])
    nc.vector.reciprocal(bb[:], n2[:])
    a2, b2, ab, c = sv("a2"), sv("b2"), sv("ab"), sv("c")
    nc.vector.tensor_mul(a2[:], a[:], a[:])
    nc.vector.tensor_mul(b2[:], bb[:], bb[:])
    nc.vector.tensor_mul(ab[:], a[:], bb[:])
    nc.vector.tensor_mul(c[:], dot[:], ab[:])
    m3c, mc = sv("m3c"), sv("mc")
    nc.vector.tensor_scalar(m3c[:], c[:], -3.0, None, A.mult)
    nc.vector.tensor_scalar(mc[:], c[:], -1.0, None, A.mult)
    A3B, AB3, A4, B4, A2B2 = sv("A3B"), sv("AB3"), sv("A4"), sv("B4"), sv("A2B2")
    nc.vector.tensor_mul(A3B[:], a2[:], ab[:])
    nc.vector.tensor_mul(AB3[:], b2[:], ab[:])
    nc.vector.tensor_mul(A4[:], a2[:], a2[:])
    nc.vector.tensor_mul(B4[:], b2[:], b2[:])
    nc.vector.tensor_mul(A2B2[:], ab[:], ab[:])
    m3cA4, m3cB4, mcA2B2, mcab = sv("m3cA4"), sv("m3cB4"), sv("mcA2B2"), sv("mcab")
    nc.vector.tensor_mul(m3cA4[:], A4[:], m3c[:])
    nc.vector.tensor_mul(m3cB4[:], B4[:], m3c[:])
    nc.vector.tensor_mul(mcA2B2[:], A2B2[:], mc[:])
    nc.vector.tensor_mul(mcab[:], ab[:], mc[:])

    # rounded copies of z1/z2 for the matmul lhsT gathers
    mmdt = f32r if use_f32r else f32
    z1r = p.zg.tile([P, D], mmdt, tag="z1r", name=f"z1r_{grp}")
    nc.vector.tensor_copy(z1r[:], z1g[:])
    z2r = p.zg.tile([P, D], mmdt, tag="z2r", name=f"z2r_{grp}")
    nc.vector.tensor_copy(z2r[:], z2g[:])

    st.update(z1g=z1g, z2g=z2g, v1z=v1z, v2z=v2z, wz=wz, a2=a2, b2=b2, ab=ab,
              c=c, m3c=m3c, A3B=A3B, AB3=AB3, m3cA4=m3cA4, m3cB4=m3cB4,
              mcA2B2=mcA2B2, mcab=mcab, z1r=z1r, z2r=z2r, wt=wt)
    return st


def _stats_phase1(p, nc, st, grp, use_f32r):
    """rhs rows R0, R1 [128b, 384] in float32r (rounded on DVE write)."""
    A = mybir.AluOpType
    mmdt = f32r if use_f32r else f32
    z1g, z2g, wt = st["z1g"], st["z2g"], st["wt"]
    A3B, AB3 = st["A3B"], st["AB3"]
    R0 = p.rpool.tile([P, KD], mmdt, tag="R0", name=f"R0_{grp}")
    R1 = p.rpool.tile([P, KD], mmdt, tag="R1", name=f"R1_{grp}")
    t0 = wt("t0")
    # k=0 (H11): r0 = m3cA4*z1 + A3B*z2 ; r1 = A3B*z1
    nc.vector.tensor_scalar(t0[:], z2g[:], A3B[:], None, A.mult)
    nc.vector.scalar_tensor_tensor(
        R0[:, 0:D], z1g[:], st["m3cA4"][:], t0[:], A.mult, A.add)
    nc.vector.tensor_scalar(R1[:, 0:D], z1g[:], A3B[:], None, A.mult)
    # k=1 (H12): r0 = A3B*z1 ; r1 = mcA2B2*z1 + AB3*z2
    nc.vector.tensor_scalar(R0[:, D:2 * D], z1g[:], A3B[:], None, A.mult)
    t1 = wt("t1")
    nc.vector.tensor_scalar(t1[:], z2g[:], AB3[:], None, A.mult)
    nc.vector.scalar_tensor_tensor(
        R1[:, D:2 * D], z1g[:], st["mcA2B2"][:], t1[:], A.mult, A.add)
    # k=2 (H22): r0 = AB3*z2 ; r1 = AB3*z1 + m3cB4*z2
    nc.vector.tensor_scalar(R0[:, 2 * D:3 * D], z2g[:], AB3[:], None, A.mult)
    t2 = wt("t2")
    nc.vector.tensor_scalar(t2[:], z2g[:], st["m3cB4"][:], None, A.mult)
    nc.vector.scalar_tensor_tensor(
        R1[:, 2 * D:3 * D], z1g[:], AB3[:], t2[:], A.mult, A.add)
    st.update(R0=R0, R1=R1)


def _stats_phase2(p, nc, st, grp):
    """Final diagonal values, batch-major [128b, 128i]."""
    A = mybir.AluOpType
    wt = st["wt"]
    v1z, v2z, wz = st["v1z"], st["v2z"], st["wz"]
    a2, b2, ab, c, m3c = st["a2"], st["b2"], st["ab"], st["c"], st["m3c"]
    twoabw = wt("twoabw")
    nc.vector.tensor_scalar(twoabw[:], wz[:], ab[:], 2.0, A.mult, A.mult)
    # d11 = a2*(c + 2ab*wz + m3c*a2*v1z)
    u1, u2 = wt("u1"), wt("u2")
    nc.vector.tensor_scalar(u1[:], v1z[:], a2[:], m3c[:], A.mult, A.mult)
    nc.vector.tensor_add(u2[:], u1[:], twoabw[:])
    d11 = p.dpool.tile([P, D], f32, tag="d11", name=f"d11_{grp}")
    nc.vector.tensor_scalar(d11[:], u2[:], c[:], a2[:], A.add, A.mult)
    # d22 = b2*(c + 2ab*wz + m3c*b2*v2z)
    u3, u4 = wt("u3"), wt("u4")
    nc.vector.tensor_scalar(u3[:], v2z[:], b2[:], m3c[:], A.mult, A.mult)
    nc.vector.tensor_add(u4[:], u3[:], twoabw[:])
    d22 = p.dpool.tile([P, D], f32, tag="d22", name=f"d22_{grp}")
    nc.vector.tensor_scalar(d22[:], u4[:], c[:], b2[:], A.add, A.mult)
    # d12 = ab*(a2*v1z + b2*v2z + mcab*wz - 1)
    w1, w2, w3 = wt("w1"), wt("w2"), wt("w3")
    nc.vector.tensor_scalar(w1[:], v1z[:], a2[:], None, A.mult)
    nc.vector.scalar_tensor_tensor(w2[:], v2z[:], b2[:], w1[:], A.mult, A.add)
    nc.vector.scalar_tensor_tensor(w3[:], wz[:], st["mcab"][:], w2[:],
                                   A.mult, A.add)
    d12 = p.dpool.tile([P, D], f32, tag="d12", name=f"d12_{grp}")
    nc.vector.tensor_scalar(d12[:], w3[:], -1.0, ab[:], A.add, A.mult)
    st.update(d11=d11, d12=d12, d22=d22)


def _stats_phase3(p, nc, st, grp):
    """Transpose diagonals into [128i, 3*128b]."""
    diagT = p.dpool.tile([P, KD], f32, tag="diagT", name=f"diagT_{grp}")
    for k, dk in enumerate([st["d11"], st["d12"], st["d22"]]):
        pt = p.tpp.tile([P, D], f32, tag="tp", name=f"tp_{grp}_{k}")
        nc.tensor.transpose(pt[:], dk[:], p.eye[:])
        nc.scalar.copy(diagT[:, k * D:(k + 1) * D], pt[:])
    st.update(diagT=diagT)


def _emit_chunk(p, nc, st, out, grp, ch, use_f32r):
    """Gathers + 32 elements (matmul/copy/diag-splice) + output DMAs."""
    mmdt = f32r if use_f32r else f32
    b0 = grp * P
    e0 = b0 + ch * G          # global element base for this chunk
    q0 = ch * G               # within-group base
    # Gather F batch rows into one partition row per (group, operand).
    # Out is a single-partition free-linear run; in is a plain slice —
    # stream orders match (b-major), dma_start only checks total size.
    ZI = p.zi.tile([P, F * D], mmdt, tag="ZI", name=f"ZI_{grp}_{ch}")
    RI = p.ri.tile([P, F * KD], mmdt, tag="RI", name=f"RI_{grp}_{ch}")
    # gathers ride the (otherwise idle) gpsimd SWDGE path so the big output
    # writes on the sync HWDGE ring can't head-of-line block the next
    # chunk's operands; the very first chunk uses the still empty sync ring
    dmae = nc.sync if (grp == 0 and ch == 0) else nc.gpsimd
    z1r, z2r, R0, R1 = st["z1r"], st["z2r"], st["R0"], st["R1"]
    for g in range(2):
        qs = q0 + g * F
        dmae.dma_start(ZI[32 * g:32 * g + 1, :], z1r[qs:qs + F, :])
        dmae.dma_start(ZI[32 * g + 1:32 * g + 2, :], z2r[qs:qs + F, :])
        dmae.dma_start(RI[32 * g:32 * g + 1, :], R0[qs:qs + F, :])
        dmae.dma_start(RI[32 * g + 1:32 * g + 2, :], R1[qs:qs + F, :])

    # staging sub-groups of GS elements: the first chunks use finer
    # granularity so the first output DMA launches earlier, and the last
    # chunk so the final drain is shorter
    ci = grp * NCH + ch
    if ci == 0:
        GS = G // 4
    elif ci in (1, 2, 3):
        GS = G // 2
    elif ci == GROUPS * NCH - 1:
        GS = G // 4
    else:
        GS = G
    diagT = st["diagT"]
    for sub in range(G // GS):
        STG = p.stage.tile([P, GS * KD], f32, tag="STG",
                           name=f"STG_{grp}_{ch}_{sub}")
        for s0 in range(GS):
            s = sub * GS + s0
            q = q0 + s            # element idx within group (0..127)
            g4, ff = s // F, s % F
            pp = 32 * g4
            lhsT = ZI[pp:pp + 2, ff * D:(ff + 1) * D]
            rhs = RI[pp:pp + 2, ff * KD:(ff + 1) * KD]
            pt = p.mmp.tile([P, KD], f32, tag="pt", name=f"pt_{grp}_{ch}_{s}")
            nc.tensor.matmul(pt[:], lhsT, rhs, start=True, stop=True)
            dst = STG[:, s0 * KD:(s0 + 1) * KD]
            nc.scalar.copy(dst, pt[:])
            # contiguous innermost j on dst/mask; data is a per-k column of
            # diagT broadcast along j (step-0 inner dim)
            datav = diagT[:].rearrange("p (k b) -> p k b", k=3)[
                :, :, q:q + 1].broadcast_to([P, 3, D])
            nc.vector.copy_predicated(dst, p.eye3[:], datav)
        bs = e0 + sub * GS
        stgv = STG[:].rearrange("p (e n) -> p e n", n=KD)
        for k in range(3):
            dram = out[k, bs:bs + GS, :, :].transpose([1, 0, 2])
            nc.sync.dma_start(dram, stgv[:, :, k * D:(k + 1) * D])


def _build_body(ctx, tc, z1, z2, out, use_f32r=True):
    nc = tc.nc
    p = _make_pools(ctx, tc)
    _emit_consts(p, nc)

    # group 0's stats run up front; each later group's stats are emitted in
    # slices between the previous group's chunks so the DVE/ACT burst is
    # amortized and no single chunk's compute exceeds its DMA drain window
    cur = _stats_phase0(p, nc, z1, z2, 0, use_f32r)
    _stats_phase1(p, nc, cur, 0, use_f32r)
    _stats_phase2(p, nc, cur, 0)
    _stats_phase3(p, nc, cur, 0)
    for grp in range(GROUPS):
        nxt = None
        for ch in range(NCH):
            _emit_chunk(p, nc, cur, out, grp, ch, use_f32r)
            if grp + 1 < GROUPS:
                if ch == 0:
                    nxt = _stats_phase0(p, nc, z1, z2, grp + 1, use_f32r)
                elif ch == 1:
                    _stats_phase1(p, nc, nxt, grp + 1, use_f32r)
                elif ch == 2:
                    _stats_phase2(p, nc, nxt, grp + 1)
                else:
                    _stats_phase3(p, nc, nxt, grp + 1)
        if nxt is not None:
            cur = nxt


def build_kernel(use_f32r=True):
    nc = bacc.Bacc("TRN2", target_bir_lowering=False, debug=False)
    z1 = nc.dram_tensor("z1", [B_SH, D], f32, kind="ExternalInput").ap()
    z2 = nc.dram_tensor("z2", [B_SH, D], f32, kind="ExternalInput").ap()
    out = nc.dram_tensor("out", [3, B_SH, D, D], f32, kind="ExternalOutput").ap()
    with tile.TileContext(nc) as tc:
        with ExitStack() as ctx:
            _build_body(ctx, tc, z1, z2, out, use_f32r=use_f32r)
    nc.compile()
    return nc


_NC_CACHE = None


def _get_nc():
    global _NC_CACHE
    if _NC_CACHE is None:
        _NC_CACHE = build_kernel()
    return _NC_CACHE


def kernel(z1, z2):
    nc = _get_nc()
    z1 = np.ascontiguousarray(np.asarray(z1, dtype=np.float32))
    z2 = np.ascontiguousarray(np.asarray(z2, dtype=np.float32))
    in_maps = [
        {"z1": z1[c * B_SH:(c + 1) * B_SH], "z2": z2[c * B_SH:(c + 1) * B_SH]}
        for c in range(N_CORES)
    ]
    res = run_bass_kernel_spmd(nc, in_maps, core_ids=list(range(N_CORES)))
    return np.concatenate([res.results[c]["out"] for c in range(N_CORES)], axis=1)
